# revision 11
# baseline (speedup 1.0000x reference)
"""Trainium2 Bass kernel for CompoundProteinInteractionPrediction GNN.

Computation (reference):
    xs = emb_fp[fingerprints]                      # [8192, 32]
    for l in 0..2:  xs = xs + A @ relu(xs @ W_l + b_l)   # A: [8192, 8192] dense
    return xs[None], emb_word[words]

Distribution (8 NeuronCores, SPMD):
    - A and xs row-sharded: core c owns rows [1024c, 1024c+1024).
    - The A slice is passed host-transposed ([8192, 1024] fp32, pure layout
      prep) and cast-DMA'd fp32->bf16 on device into a SBUF-resident A^T.
    - Per layer: each core computes hs for its own rows, AllGathers hs
      (bf16, 64KB per rank), then accumulates yT = (A_local @ hs)^T on the
      PE with hs tiles stationary and A^T tiles moving; residual added into
      the transposed state xsT on DVE.
    - Embedding lookups use indirect DMA gathers; tables replicated.
"""

import os
import sys

for _p in ("/opt/trn_rl_repo", "/root/.axon_site/_ro/trn_rl_repo"):
    if os.path.isdir(_p) and _p not in sys.path:
        sys.path.insert(0, _p)

import numpy as np

import concourse.bass as bass
import concourse.bacc as bacc
import concourse.mybir as mybir
import concourse.tile as tile
from concourse.bass_utils import run_bass_kernel_spmd
from concourse.masks import make_identity

NCORES = 8
N = 8192            # atoms
D = 32              # hidden dim
LAYERS = 3
M = N // NCORES     # 1024 rows per core
P = 128
NKT = N // P        # 64 contraction tiles
NMT = M // P        # 8 local row tiles
S = 4096            # words
SL = S // NCORES    # 512 words per core
N_FP = 100000
N_WORD = 10000

FP32 = mybir.dt.float32
BF16 = mybir.dt.bfloat16
INT32 = mybir.dt.int32


def _build(sim_single_core: bool = False, ablate: str = ""):
    # sim_single_core: build a 1-core variant with the AllGather replaced by
    # a local DRAM copy, for TimelineSim cost-model analysis (no collectives).
    # ablate: comma-set of {"layers","aload"} to skip sections (sim only).
    skip_layers = "layers" in ablate
    skip_aload = "aload" in ablate
    ndev = 1 if sim_single_core else NCORES
    nc = bacc.Bacc("TRN2", target_bir_lowering=False, debug=False, num_devices=ndev)

    at = nc.declare_dram_parameter("at", [N, M], FP32, isOutput=False)
    emb_fp = nc.declare_dram_parameter("emb_fp", [N_FP, D], FP32, isOutput=False)
    emb_word = nc.declare_dram_parameter("emb_word", [N_WORD, D], FP32, isOutput=False)
    fpi = nc.declare_dram_parameter("fpi", [M], INT32, isOutput=False)
    wdi = nc.declare_dram_parameter("wdi", [SL], INT32, isOutput=False)
    wg = nc.declare_dram_parameter("wg", [LAYERS, D, D], FP32, isOutput=False)
    bg = nc.declare_dram_parameter("bg", [LAYERS, D], FP32, isOutput=False)

    out_x = nc.declare_dram_parameter("out_x", [D, M], FP32, isOutput=True)
    out_wv = nc.declare_dram_parameter("out_wv", [SL, D], FP32, isOutput=True)

    with tile.TileContext(nc) as tc:
        with (
            tc.tile_pool(name="persist", bufs=1) as persist,
            tc.tile_pool(name="work", bufs=2) as work,
            tc.tile_pool(name="hsps", bufs=2, space="PSUM") as hsps,
            tc.tile_pool(name="ytps", bufs=2, space="PSUM") as ytps,
            tc.tile_pool(name="dram", bufs=3, space="DRAM") as dram,
        ):
            # ---- small constant loads (HWDGE, ahead of everything) ----
            ident = persist.tile([P, P], FP32)
            make_identity(nc, ident[:])
            # S[p, n] = 1 iff p % 32 == n: PE-side reduction over 4 col groups
            s_sb = persist.tile([P, D], FP32)
            for j in range(4):
                nc.sync.dma_start(
                    out=s_sb[32 * j : 32 * (j + 1), :], in_=ident[0:D, 0:D]
                )

            w33 = persist.tile([D + 1, LAYERS, D], FP32)
            nc.sync.dma_start(out=w33[0:D, :, :], in_=wg[:].rearrange("l d n -> d l n"))
            nc.sync.dma_start(out=w33[D : D + 1, :, :], in_=bg[:])

            fpi_sb = persist.tile([P, NMT], INT32)
            nc.sync.dma_start(out=fpi_sb[:], in_=fpi[:].rearrange("(t p) -> p t", p=P))
            wdi_sb = persist.tile([P, SL // P], INT32)
            nc.sync.dma_start(out=wdi_sb[:], in_=wdi[:].rearrange("(t p) -> p t", p=P))

            # ---- embedding gathers (SWDGE indirect, before the big A stream) ----
            xs_nat = persist.tile([P, NMT, D], FP32)
            for t in range(NMT):
                nc.gpsimd.indirect_dma_start(
                    out=xs_nat[:, t, :],
                    out_offset=None,
                    in_=emb_fp[:],
                    in_offset=bass.IndirectOffsetOnAxis(ap=fpi_sb[:, t : t + 1], axis=0),
                )
            wv_sb = persist.tile([P, SL // P, D], FP32)
            for t in range(SL // P):
                nc.gpsimd.indirect_dma_start(
                    out=wv_sb[:, t, :],
                    out_offset=None,
                    in_=emb_word[:],
                    in_offset=bass.IndirectOffsetOnAxis(ap=wdi_sb[:, t : t + 1], axis=0),
                )
            nc.sync.dma_start(
                out=out_wv[:].rearrange("(t p) d -> p t d", p=P), in_=wv_sb[:]
            )

            # ---- state: xsT [33, 1024] fp32, row 32 = ones (bias trick) ----
            xsT = persist.tile([D + 1, M], FP32)
            nc.gpsimd.memset(xsT[D : D + 1, :], 1.0)
            for t in range(NMT):
                tr_ps = hsps.tile([D, P], FP32, space="PSUM", tag="sm")
                nc.tensor.transpose(out=tr_ps[:], in_=xs_nat[:, t, :], identity=ident[:])
                nc.vector.tensor_copy(out=xsT[0:D, t * P : (t + 1) * P], in_=tr_ps[:])

            # ---- A^T resident load: cast-DMA fp32 -> bf16, one DMA per k-tile ----
            at_bf = persist.tile([P, NKT, M], BF16)
            if not skip_aload:
                for t in range(NKT):
                    nc.gpsimd.dma_start(
                        out=at_bf[:, t, :], in_=at[t * P : (t + 1) * P, :]
                    )

            # ---- layers ----
            for l in range(0 if skip_layers else (1 if "one" in ablate else LAYERS)):
                # hs for own rows: [128, 32] bf16 per local tile
                hs_own = work.tile([P, NMT, D], BF16, tag="hs_own")
                for t in range(NMT):
                    h_ps = hsps.tile([P, D], FP32, space="PSUM", tag="sm")
                    nc.tensor.matmul(
                        h_ps[:],
                        lhsT=xsT[:, t * P : (t + 1) * P],
                        rhs=w33[:, l, :],
                        start=True,
                        stop=True,
                    )
                    nc.scalar.activation(
                        out=hs_own[:, t, :],
                        in_=h_ps[:],
                        func=mybir.ActivationFunctionType.Relu,
                    )

                # AllGather hs across ranks (bf16)
                cc_in = dram.tile([M, D], BF16, tag="cc_in")
                cc_out = dram.tile([N, D], BF16, tag="cc_out")
                nc.sync.dma_start(
                    out=cc_in[:].rearrange("(t p) n -> p t n", p=P), in_=hs_own[:]
                )
                if sim_single_core:
                    nc.sync.dma_start(out=cc_out[0:M, :], in_=cc_in[:])
                else:
                    nc.gpsimd.collective_compute(
                        "AllGather",
                        mybir.AluOpType.bypass,
                        replica_groups=[list(range(NCORES))],
                        ins=[cc_in[:]],
                        outs=[cc_out[:]],
                    )
                hs_all = work.tile([P, NKT, D], BF16, tag="hs_all")
                cc_r = cc_out[:].rearrange("(t p) n -> p t n", p=P)
                q4 = NKT // 4
                for q in range(4):
                    nc.sync.dma_start(
                        out=hs_all[:, q * q4 : (q + 1) * q4, :],
                        in_=cc_r[:, q * q4 : (q + 1) * q4, :],
                    )

                # yT = (A_local @ hs)^T, 4x PE column tiling: group j takes
                # k-tiles t = 4i+j, accumulating into psum partitions 32j:32j+32.
                yt0 = ytps.tile([P, 512], FP32, space="PSUM", tag="yt0")
                yt1 = ytps.tile([P, 512], FP32, space="PSUM", tag="yt1")
                yts = [yt0, yt1]
                # half-major order: finish columns [0:512] first so its
                # reduction/residual overlaps the second half's matmuls.
                NQ = NKT // 4
                for h in range(2):
                    for i in range(NQ):
                        for j in range(4):
                            t = 4 * i + j
                            nc.tensor.matmul(
                                yts[h][32 * j : 32 * (j + 1), :],
                                lhsT=hs_all[:, t, :],
                                rhs=at_bf[:, t, 512 * h : 512 * (h + 1)],
                                start=(i == 0),
                                stop=(i == NQ - 1),
                                tile_position=(0, 32 * j),
                            )
                    psb = work.tile([P, 512], FP32, tag="psb")
                    nc.vector.tensor_copy(out=psb[:], in_=yts[h][:])
                    rps = hsps.tile([D, 512], FP32, space="PSUM", tag="sm")
                    nc.tensor.matmul(
                        rps[:], lhsT=s_sb[:], rhs=psb[:], start=True, stop=True
                    )
                    nc.vector.tensor_add(
                        out=xsT[0:D, 512 * h : 512 * (h + 1)],
                        in0=xsT[0:D, 512 * h : 512 * (h + 1)],
                        in1=rps[:],
                    )

            nc.sync.dma_start(out=out_x[:], in_=xsT[0:D, :])

    nc.compile()
    return nc


_NC = None


def _get_nc():
    global _NC
    if _NC is None:
        _NC = _build()
    return _NC


def _run(in_maps, trace=False, **kwargs):
    nc = _get_nc()
    return run_bass_kernel_spmd(nc, in_maps, list(range(NCORES)), trace=trace, **kwargs)


def make_in_maps(fingerprints, adjacency, words, emb_fp, emb_word, Wg, bg):
    adjacency = np.asarray(adjacency, dtype=np.float32)
    emb_fp = np.ascontiguousarray(np.asarray(emb_fp, dtype=np.float32))
    emb_word = np.ascontiguousarray(np.asarray(emb_word, dtype=np.float32))
    Wg = np.ascontiguousarray(np.asarray(Wg, dtype=np.float32))
    bg = np.ascontiguousarray(np.asarray(bg, dtype=np.float32))
    fingerprints = np.asarray(fingerprints).astype(np.int32)
    words = np.asarray(words).astype(np.int32)

    in_maps = []
    for c in range(NCORES):
        rows = slice(c * M, (c + 1) * M)
        in_maps.append(
            {
                "at": np.ascontiguousarray(adjacency[rows, :].T),
                "emb_fp": emb_fp,
                "emb_word": emb_word,
                "fpi": np.ascontiguousarray(fingerprints[rows]),
                "wdi": np.ascontiguousarray(words[c * SL : (c + 1) * SL]),
                "wg": Wg,
                "bg": bg,
            }
        )
    return in_maps


def assemble(results):
    xsT = np.concatenate([results[c]["out_x"] for c in range(NCORES)], axis=1)
    compound = np.ascontiguousarray(xsT.T)[None].astype(np.float32)
    wv = np.concatenate([results[c]["out_wv"] for c in range(NCORES)], axis=0).astype(
        np.float32
    )
    return compound, wv


def kernel(fingerprints, adjacency, words, emb_fp, emb_word, Wg, bg):
    in_maps = make_in_maps(
        fingerprints, adjacency, words, emb_fp, emb_word, Wg, bg
    )
    res = _run(in_maps).results
    return assemble(res)
